# revision 17
# baseline (speedup 1.0000x reference)
"""Trainium2 Bass kernel for nn_ActivationPNALayer (PNA-style GNN message passing).

Strategy (8 NeuronCores, SPMD):
  - 1D graph partition by destination node; nodes are globally sorted by
    in-degree and dealt round-robin across cores (rank r -> core r%8), so every
    core's block b spans the same global degree range -> minimal slot padding
    and perfect load balance.
  - Host-side partitioning/layout: edges laid out as a padded
    [128-node-block x K_b] slot grid in k-OUTER (k, j, d) order; halo features
    h[src]*norm[src] are materialized into a per-edge fp16 stream at partition
    time (ghost-feature materialization).
  - Device: streams hn-slots + ef-slots, forms w = hn[src]+ef (fp16, DVE 2x),
    reduces sum/sumsq/max/min over the slot axis per node via pairwise fp16
    tensor_tensor trees. k-outer layout keeps the packed (j,d) axis innermost
    at every tree level, so every level runs in the DVE 2x fp16 mode
    (tensor_reduce has no fast mode; strided single-column ops are 2-4x
    slower than packed ones). Per-node scalars (1/deg, wn, c1, npad, valid)
    are folded on the host into expanded [node,D] fp16 side streams so the
    tail has no stride-0 broadcast multiplies.
  - Both BatchNorms use PE for the [2 x D] statistics with a cross-core
    AllReduce. Pads duplicate each node's first edge (max/min-safe); sums are
    corrected with npad * w0 where w0 is sliced from slot 0 on device.

Math identity used (the reference's concat/mean tower collapses):
  z_e = w_e + hin[dst],  w_e = hin[src_e] + ef_e
  mean_z = mean_w + hin; max_z = max_w + hin; min_z = min_w + hin; std_z = std_w
  ht = hin*(1/13 + 3*wn) + (mean_w+max_w+min_w+std_w) * wn,
  wn = (1 + logd/A + A/logd)/13
  out = BN2(relu(BN1(ht)) * norm)   [relu(u*norm) = relu(u)*norm since norm > 0]
"""

import os
import sys
from contextlib import ExitStack

import numpy as np

for _p in ("/opt/trn_rl_repo",):
    if os.path.isdir(_p) and _p not in sys.path:
        sys.path.insert(0, _p)

import concourse.bacc as bacc
import concourse.bass as bass
import concourse.mybir as mybir
import concourse.tile as tile
from concourse.bass_utils import run_bass_kernel_spmd

AluOp = mybir.AluOpType
ActFn = mybir.ActivationFunctionType
Axis = mybir.AxisListType
F32 = mybir.dt.float32
F16 = mybir.dt.float16
NPF16 = np.float16

EPS_STD = 1e-5
EPS_BN = 1e-5
AVG_D_LOG = float(np.log(17.0))
N_CORES = 8
FDBUD_DEFAULT = 10240   # max free-dim elements of one slot-stream tile
SQ_ON_GPSIMD = True    # A/B: run the sumsq tree on the Pool engine


# ----------------------------------------------------------------------------
# Host-side partition / layout prep
# ----------------------------------------------------------------------------

def _prep(h, ef, norm, gamma1, beta1, gamma2, beta2, src, dst, C,
          fdbud=FDBUD_DEFAULT):
    h = np.asarray(h, np.float32)
    ef = np.ascontiguousarray(np.asarray(ef, np.float32))
    norm = np.asarray(norm, np.float32).reshape(-1, 1)
    src = np.asarray(src).astype(np.int64)
    dst = np.asarray(dst).astype(np.int64)
    N, D = h.shape
    NS = N // C
    NPAD = ((NS + 127) // 128) * 128
    NB = NPAD // 128
    CH = max(d for d in range(1, min(NB, 49) + 1) if NB % d == 0)
    NCH = NB // CH

    hn = h * norm  # ghost/halo feature table materialized at partition time

    deg = np.bincount(dst, minlength=N).astype(np.int64)
    order = np.argsort(dst, kind="stable")
    starts = np.zeros(N + 1, np.int64)
    starts[1:] = np.cumsum(deg)

    # striped degree-sorted assignment: rank r -> core r%C, position r//C
    grank = np.argsort(-deg, kind="stable")
    nids = np.empty((C, NPAD), np.int64)
    for c in range(C):
        ids = grank[c::C]
        if NPAD > NS:
            ids = np.concatenate([ids, np.full(NPAD - NS, ids[-1], np.int64)])
        nids[c] = ids
    degs_sorted = deg[nids]                       # [C, NPAD]

    Kb = degs_sorted.reshape(C, NB, 128).max(axis=(0, 2))
    Kb = np.maximum(Kb, 1).astype(np.int64)

    # block groups per chunk: uniform K per group, free-dim budget fdbud.
    groups = []          # [NCH][(j0, nb, K, chunk-local coloff)]
    Kb_eff = np.zeros(NB, np.int64)   # actual slots per block (group-uniform)
    TOTF_ch = []
    for ch in range(NCH):
        j = 0
        gl = []
        coloff = 0
        while j < CH:
            K = int(Kb[ch * CH + j])
            if K > 1:
                K += K & 1          # even K: kills the level-1 tree fold ops
            nb = int(min(CH - j, max(1, fdbud // (D * K))))
            gl.append((j, nb, K, coloff))
            Kb_eff[ch * CH + j: ch * CH + j + nb] = K
            coloff += nb * D * K
            j += nb
        groups.append(gl)
        TOTF_ch.append(coloff)

    gbv = np.concatenate(
        [np.asarray(x, np.float32).reshape(-1) for x in (gamma1, beta1, gamma2, beta2)]
    ).reshape(1, 4 * D)

    # per-node folded scalars (f64 on host, stored fp16 expanded over D)
    degf = deg.astype(np.float64)
    logd = np.log(degf + 1.0)
    wn_n = (1.0 + logd / AVG_D_LOG + AVG_D_LOG / logd) / 13.0
    c1_n = 1.0 / 13.0 + 3.0 * wn_n
    invd_n = 1.0 / degf

    in_maps = []
    for c in range(C):
        nid_grid = nids[c].reshape(NB, 128)          # [NB, 128] global ids

        # flat per-chunk streams: group g occupies [128*off, 128*(off+F)) as a
        # [128, F] C-order block; within a group the order is (k, j, d) so the
        # packed (j, d) axis stays innermost at every tree level.
        stream = {}
        for ch in range(NCH):
            hns = np.empty(128 * TOTF_ch[ch], NPF16)
            efs = np.empty(128 * TOTF_ch[ch], NPF16)
            for (j0, nb, K, off) in groups[ch]:
                b0 = ch * CH + j0
                n = nid_grid[b0:b0 + nb]                       # [nb, 128]
                s = starts[n][:, :, None]                      # [nb, 128, 1]
                ks = np.arange(K)[None, None, :]
                kk = np.where(ks < deg[n][:, :, None], ks, 0)  # pads -> slot 0
                eids = order[s + kk]                           # [nb, 128, K]
                sidx = src[eids]
                # (nb, 128, K, D) -> (128, K, nb, D)
                hg = hn[sidx].transpose(1, 2, 0, 3).reshape(128, nb * D * K)
                eg = ef[eids].transpose(1, 2, 0, 3).reshape(128, nb * D * K)
                goff = 128 * off
                hns[goff:goff + 128 * nb * D * K] = hg.astype(NPF16).reshape(-1)
                efs[goff:goff + 128 * nb * D * K] = eg.astype(NPF16).reshape(-1)
            stream[f"hns{ch}"] = hns
            stream[f"efs{ch}"] = efs

        meta = nid_grid.T                            # [128, NB]
        sortedidx = np.arange(NPAD).reshape(NB, 128).T
        validf = (sortedidx < NS).astype(np.float64)
        normv = (norm[meta, 0] * validf)             # norm, zeroed for pads
        npadm = (Kb_eff[None, :] - deg[meta]).astype(np.float64)

        def expand(vals128xNB):  # -> [128, NB*D] fp16
            return np.ascontiguousarray(
                np.repeat(vals128xNB[:, :, None], D, axis=2)
                .reshape(128, NB * D).astype(NPF16))

        # hselfC1 = hn[node]*c1*valid  [128, NB*D]
        hselfC1 = (hn[nid_grid] * (c1_n[nid_grid] * validf.T)[:, :, None]) \
            .transpose(1, 0, 2).reshape(128, NB * D)

        in_maps.append(dict(
            **stream,
            hselfc1=np.ascontiguousarray(hselfC1.astype(NPF16)),
            wnd=expand(wn_n[meta] * validf),
            invd=expand(invd_n[meta]),
            npd=expand(npadm),
            nrmd=expand(normv),
            gbv=gbv,
        ))

    layout = dict(N=N, D=D, C=C, NS=NS, NPAD=NPAD, NB=NB, CH=CH, NCH=NCH,
                  groups=groups, TOTF_ch=TOTF_ch, Kb=[int(k) for k in Kb],
                  nids=nids)
    return in_maps, layout


# ----------------------------------------------------------------------------
# Device program
# ----------------------------------------------------------------------------

def _build(layout):
    N = layout["N"]; D = layout["D"]; C = layout["C"]
    NB = layout["NB"]; CH = layout["CH"]; NCH = layout["NCH"]
    groups = layout["groups"]; TOTF_ch = layout["TOTF_ch"]

    nc = bacc.Bacc("TRN2", target_bir_lowering=False, debug=False,
                   enable_asserts=False, num_devices=C)

    hns_d = [nc.dram_tensor(f"hns{ch}", [128 * TOTF_ch[ch]], F16,
                            kind="ExternalInput") for ch in range(NCH)]
    efs_d = [nc.dram_tensor(f"efs{ch}", [128 * TOTF_ch[ch]], F16,
                            kind="ExternalInput") for ch in range(NCH)]
    hselfc1_d = nc.dram_tensor("hselfc1", [128, NB * D], F16, kind="ExternalInput")
    wnd_d = nc.dram_tensor("wnd", [128, NB * D], F16, kind="ExternalInput")
    invd_d = nc.dram_tensor("invd", [128, NB * D], F16, kind="ExternalInput")
    npd_d = nc.dram_tensor("npd", [128, NB * D], F16, kind="ExternalInput")
    nrmd_d = nc.dram_tensor("nrmd", [128, NB * D], F16, kind="ExternalInput")
    gbv_d = nc.dram_tensor("gbv", [1, 4 * D], F32, kind="ExternalInput")
    out_d = nc.dram_tensor("out", [128, NB * D], F16, kind="ExternalOutput")

    rg = [list(range(C))]

    with tile.TileContext(nc) as tc, ExitStack() as stack:
        res = stack.enter_context(tc.tile_pool(name="res", bufs=1))
        slotp = stack.enter_context(tc.tile_pool(name="slot", bufs=2))
        scrp = stack.enter_context(tc.tile_pool(name="scr", bufs=1))
        chkp = stack.enter_context(tc.tile_pool(name="chk", bufs=2))
        tailp = stack.enter_context(tc.tile_pool(name="tail", bufs=1))
        psp = stack.enter_context(tc.tile_pool(name="ps", bufs=1, space="PSUM"))
        drp = stack.enter_context(tc.tile_pool(name="dr", bufs=1, space="DRAM"))

        # resident tiles
        gb_t = res.tile([1, 4 * D], F32)
        nc.sync.dma_start(gb_t[:], gbv_d.ap())

        ones_colf = res.tile([128, 1], F32)
        ones_row = res.tile([1, 128], F32)
        eps_t = res.tile([128, 1], F32)
        nc.vector.memset(ones_colf[:], 1.0)
        nc.vector.memset(ones_row[:], 1.0)
        nc.vector.memset(eps_t[:], EPS_STD)

        HT = res.tile([128, NB * D], F16)     # ht, then y in place

        # warm up the collective path early (hidden under phase-1 streaming)
        # so the two real AllReduces run at steady-state latency (~9us vs ~37us)
        wi = drp.tile([1, 2 * D], F32, tag="warmin")
        wo = drp.tile([1, 2 * D], F32, tag="warmout")
        nc.sync.dma_start(wi[:], gb_t[:, 0:2 * D])
        nc.gpsimd.collective_compute(
            "AllReduce", AluOp.add, replica_groups=rg,
            ins=[wi[:].opt()], outs=[wo[:].opt()])



        def bcast_node(ap128xCH):
            return ap128xCH.to_broadcast([128, CH, D])

        def bcast_feat(ap128xD):
            return ap128xD.rearrange("p (o d) -> p o d", o=1).to_broadcast([128, CH, D])

        def bn_stats_chunk(htc, pbn, first):
            """Per-chunk BN partial: fp16 trees over the CH chunk columns,
            accumulated into [128, 2D] f32; one AllReduce per BN happens at
            the phase boundary (mid-phase collectives block the gpsimd queue
            and can cascade into DVE stalls)."""
            SQH = tailp.tile([128, CH * D], F16, tag="npdc")
            nc.scalar.activation(out=SQH[:], in_=htc, func=ActFn.Square)
            ht3 = htc.rearrange("p (c d) -> p c d", c=CH)
            sq3 = SQH[:].rearrange("p (c d) -> p c d", c=CH)
            Tb = tailp.tile([128, (CH // 2) * D], F16, tag="bntree")
            Tb3 = Tb[:].rearrange("p (c d) -> p c d", c=CH // 2)
            if first:
                tree(nc.vector, ht3, CH, AluOp.add,
                     pbn[:, 0:D].rearrange("p (c d) -> p c d", c=1), Tb3)
                tree(nc.vector, sq3, CH, AluOp.add,
                     pbn[:, D:2 * D].rearrange("p (c d) -> p c d", c=1), sq3)
            else:
                pacc = tailp.tile([128, 2 * D], F32, tag="pacc")
                tree(nc.vector, ht3, CH, AluOp.add,
                     pacc[:, 0:D].rearrange("p (c d) -> p c d", c=1), Tb3)
                tree(nc.vector, sq3, CH, AluOp.add,
                     pacc[:, D:2 * D].rearrange("p (c d) -> p c d", c=1), sq3)
                nc.vector.tensor_tensor(out=pbn[:], in0=pbn[:], in1=pacc[:],
                                        op=AluOp.add)

        def tree(eng, cur3, s, op, acc3, Tv):
            """Pairwise reduce [128, s, nbD] over axis 1 into acc3
            ([128, 1, nbD]); Tv is the scratch view [128, s//2, nbD] (may
            alias cur3's buffer for in-place trees). k-outer layout keeps the
            innermost axis packed at every level -> DVE 2x fp16 mode."""
            if s == 1:
                nc.vector.tensor_copy(out=acc3, in_=cur3[:, 0:1, :])
                return
            while s > 1:
                hh = s // 2
                out = acc3 if hh == 1 else Tv[:, 0:hh, :]
                eng.tensor_tensor(out=out, in0=cur3[:, 0:hh, :],
                                  in1=cur3[:, hh:2 * hh, :], op=op)
                if s % 2:
                    eng.tensor_tensor(out=out[:, 0:1, :], in0=out[:, 0:1, :],
                                      in1=cur3[:, 2 * hh:s, :], op=op)
                cur3 = out
                s = hh

        # ================= phase 1: stream slots, reduce, tower =================
        pbn1 = res.tile([128, 2 * D], F32, name="pbn1")
        pbn2 = res.tile([128, 2 * D], F32, name="pbn2")
        for ch in range(NCH):
            cols = slice(ch * CH * D, (ch + 1) * CH * D)
            colsn = slice(ch * CH, (ch + 1) * CH)
            sumC = chkp.tile([128, CH * D], F16, tag="sumC")
            maxC = chkp.tile([128, CH * D], F16, tag="maxC")
            minC = chkp.tile([128, CH * D], F16, tag="minC")
            sqC = chkp.tile([128, CH * D], F16, tag="sqC")
            w0c = chkp.tile([128, CH * D], F16, tag="w0c")

            for (j0, nb, K, off) in groups[ch]:
                F = nb * D * K
                goff = 128 * off
                ef_view = efs_d[ch].ap()[goff:goff + 128 * F].rearrange(
                    "(p f) -> p f", p=128)
                hn_view = hns_d[ch].ap()[goff:goff + 128 * F].rearrange(
                    "(p f) -> p f", p=128)
                W = slotp.tile([128, F], F16, tag="w")
                nc.sync.dma_start(W[:], ef_view)
                if CCE_ADD:
                    inst = nc.sync.dma_start(W[:], hn_view)
                    inst.ins.cce_op = AluOp.add
                    SQT = slotp.tile([128, F], F16, tag="hn")
                else:
                    HNS = slotp.tile([128, F], F16, tag="hn")
                    nc.sync.dma_start(HNS[:], hn_view)
                    nc.vector.tensor_tensor(out=W[:], in0=W[:], in1=HNS[:],
                                            op=AluOp.add)
                    SQT = HNS
                Wv = W[:].rearrange("p (k f) -> p k f", k=K)
                osl = slice(j0 * D, (j0 + nb) * D)

                def acc3(t):
                    return t[:, osl].rearrange("p (k f) -> p k f", k=1)

                # w0 slice (first real edge) on Act; SQ overwrites HNS buffer
                nc.scalar.activation(out=acc3(w0c), in_=Wv[:, 0:1, :],
                                     func=ActFn.Copy)
                nc.scalar.activation(out=SQT[:], in_=W[:], func=ActFn.Square)
                SQv = SQT[:].rearrange("p (k f) -> p k f", k=K)

                if K > 1:
                    T = scrp.tile([128, nb * D * (K // 2)], F16, tag="t")
                    Tv = T[:].rearrange("p (k f) -> p k f", k=K // 2)
                else:
                    Tv = None
                tree(nc.vector, Wv, K, AluOp.add, acc3(sumC), Tv)
                tree(nc.vector, Wv, K, AluOp.max, acc3(maxC), Tv)
                tree(nc.vector, Wv, K, AluOp.min, acc3(minC), Tv)
                sq_eng = nc.gpsimd if SQ_ON_GPSIMD else nc.vector
                tree(sq_eng, SQv, K, AluOp.add, acc3(sqC), SQv)

            # ---- chunk tail (all packed fp16, no broadcasts) ----
            hsc = tailp.tile([128, CH * D], F16, tag="hsc")
            nc.sync.dma_start(hsc[:], hselfc1_d.ap()[:, cols])
            wndc = tailp.tile([128, CH * D], F16, tag="wndc")
            nc.sync.dma_start(wndc[:], wnd_d.ap()[:, cols])
            invdc = tailp.tile([128, CH * D], F16, tag="invdc")
            nc.sync.dma_start(invdc[:], invd_d.ap()[:, cols])
            npdc = tailp.tile([128, CH * D], F16, tag="npdc")
            nc.sync.dma_start(npdc[:], npd_d.ap()[:, cols])

            tmp = tailp.tile([128, CH * D], F16, tag="tmp")

            # mean = (sumC - npd*w0)*invd  (into sumC)
            nc.vector.tensor_tensor(out=tmp[:], in0=w0c[:], in1=npdc[:],
                                    op=AluOp.mult)
            nc.vector.tensor_tensor(out=sumC[:], in0=sumC[:], in1=tmp[:],
                                    op=AluOp.subtract)
            nc.vector.tensor_tensor(out=sumC[:], in0=sumC[:], in1=invdc[:],
                                    op=AluOp.mult)
            # ex2 = (sqC - npd*w0^2)*invd  (into sqC); npd*w0^2 = tmp*w0
            nc.vector.tensor_tensor(out=tmp[:], in0=tmp[:], in1=w0c[:],
                                    op=AluOp.mult)
            nc.vector.tensor_tensor(out=sqC[:], in0=sqC[:], in1=tmp[:],
                                    op=AluOp.subtract)
            nc.vector.tensor_tensor(out=sqC[:], in0=sqC[:], in1=invdc[:],
                                    op=AluOp.mult)
            # std = sqrt(relu(ex2 - mean^2) + eps)  (into sqC)
            nc.scalar.activation(out=tmp[:], in_=sumC[:], func=ActFn.Square)
            nc.vector.tensor_tensor(out=sqC[:], in0=sqC[:], in1=tmp[:],
                                    op=AluOp.subtract)
            nc.scalar.activation(out=sqC[:], in_=sqC[:], func=ActFn.Relu)
            nc.scalar.activation(out=sqC[:], in_=sqC[:], func=ActFn.Sqrt,
                                 bias=eps_t[:])

            # S = mean + max + min + std (into maxC)
            nc.vector.tensor_tensor(out=maxC[:], in0=maxC[:], in1=minC[:], op=AluOp.add)
            nc.vector.tensor_tensor(out=maxC[:], in0=maxC[:], in1=sumC[:], op=AluOp.add)
            nc.vector.tensor_tensor(out=maxC[:], in0=maxC[:], in1=sqC[:], op=AluOp.add)

            # ht = hself*c1*valid + S*wn*valid
            htc = HT[:, cols]
            nc.vector.tensor_tensor(out=maxC[:], in0=maxC[:], in1=wndc[:],
                                    op=AluOp.mult)
            nc.vector.tensor_tensor(out=htc, in0=hsc[:], in1=maxC[:], op=AluOp.add)
            bn_stats_chunk(htc, pbn1, ch == 0)

        # ================= BN finalize helper =================
        def bn_finalize(pbn, g_off, b_off, cc_tag):
            pcmb = psp.tile([1, 2 * D], F32, tag="pscmb", space="PSUM")
            nc.tensor.matmul(out=pcmb[:, 0:D], lhsT=ones_colf[:],
                             rhs=pbn[:, 0:D], start=True, stop=True)
            nc.tensor.matmul(out=pcmb[:, D:2 * D], lhsT=ones_colf[:],
                             rhs=pbn[:, D:2 * D], start=True, stop=True)
            stats = res.tile([1, 2 * D], F32, tag=f"st{cc_tag}")
            nc.vector.tensor_copy(out=stats[:], in_=pcmb[:])
            cin = drp.tile([1, 2 * D], F32, tag=f"cin{cc_tag}")
            cout = drp.tile([1, 2 * D], F32, tag=f"cout{cc_tag}")
            nc.sync.dma_start(cin[:], stats[:])
            nc.gpsimd.collective_compute(
                "AllReduce", AluOp.add, replica_groups=rg,
                ins=[cin[:].opt()], outs=[cout[:].opt()])
            ar = res.tile([1, 2 * D], F32, tag=f"ar{cc_tag}")
            nc.sync.dma_start(ar[:], cout[:])
            mean = res.tile([1, D], F32, tag=f"mean{cc_tag}")
            nc.vector.tensor_scalar(out=mean[:], in0=ar[:, 0:D], scalar1=1.0 / N,
                                    scalar2=None, op0=AluOp.mult)
            var = res.tile([1, D], F32, tag=f"var{cc_tag}")
            nc.vector.tensor_scalar(out=var[:], in0=ar[:, D:2 * D], scalar1=1.0 / N,
                                    scalar2=None, op0=AluOp.mult)
            msq = res.tile([1, D], F32, tag=f"msq{cc_tag}")
            nc.scalar.activation(out=msq[:], in_=mean[:], func=ActFn.Square)
            nc.vector.tensor_tensor(out=var[:], in0=var[:], in1=msq[:],
                                    op=AluOp.subtract)
            nc.scalar.activation(out=var[:], in_=var[:], func=ActFn.Sqrt,
                                 bias=eps_t[:1, :])
            rstd = res.tile([1, D], F32, tag=f"rstd{cc_tag}")
            nc.vector.reciprocal(rstd[:], var[:])
            sc = res.tile([1, 2 * D], F32, tag=f"sc{cc_tag}")
            nc.vector.tensor_tensor(out=sc[:, 0:D], in0=gb_t[:, g_off:g_off + D],
                                    in1=rstd[:], op=AluOp.mult)
            nc.vector.tensor_tensor(out=sc[:, D:2 * D], in0=mean[:], in1=sc[:, 0:D],
                                    op=AluOp.mult)
            nc.vector.tensor_tensor(out=sc[:, D:2 * D], in0=gb_t[:, b_off:b_off + D],
                                    in1=sc[:, D:2 * D], op=AluOp.subtract)
            psb = psp.tile([128, 2 * D], F32, tag="psbc", space="PSUM")
            nc.tensor.matmul(out=psb[:], lhsT=ones_row[:], rhs=sc[:], start=True,
                             stop=True)
            scb = res.tile([128, 2 * D], F16, tag=f"scb{cc_tag}")
            nc.vector.tensor_copy(out=scb[:], in_=psb[:])
            return scb

        scb1 = bn_finalize(pbn1, 0, D, "1")

        # ================= phase 2: y = relu(BN1(ht)) * norm =================
        for ch in range(NCH):
            cols = slice(ch * CH * D, (ch + 1) * CH * D)
            colsn = slice(ch * CH, (ch + 1) * CH)
            htc = HT[:, cols]
            ht3 = htc.rearrange("p (c d) -> p c d", c=CH)
            U = tailp.tile([128, CH * D], F16, tag="tmp")
            u3 = U[:].rearrange("p (c d) -> p c d", c=CH)
            nc.vector.tensor_tensor(out=u3, in0=ht3, in1=bcast_feat(scb1[:, 0:D]),
                                    op=AluOp.mult)
            nc.vector.tensor_tensor(out=u3, in0=u3, in1=bcast_feat(scb1[:, D:2 * D]),
                                    op=AluOp.add)
            nc.scalar.activation(out=U[:], in_=U[:], func=ActFn.Relu)
            # nrmd = norm * valid expanded (host): zeroes pad nodes for BN2 stats
            nrc = tailp.tile([128, CH * D], F16, tag="invdc")
            nc.sync.dma_start(nrc[:], nrmd_d.ap()[:, cols])
            nc.vector.tensor_tensor(out=htc, in0=U[:], in1=nrc[:],
                                    op=AluOp.mult)
            bn_stats_chunk(htc, pbn2, ch == 0)

        scb2 = bn_finalize(pbn2, 2 * D, 3 * D, "2")

        # ================= phase 3: out = BN2(y) =================
        for ch in range(NCH):
            cols = slice(ch * CH * D, (ch + 1) * CH * D)
            htc = HT[:, cols]
            ht3 = htc.rearrange("p (c d) -> p c d", c=CH)
            O = tailp.tile([128, CH * D], F16, tag="tmp")
            o3 = O[:].rearrange("p (c d) -> p c d", c=CH)
            nc.vector.tensor_tensor(out=o3, in0=ht3, in1=bcast_feat(scb2[:, 0:D]),
                                    op=AluOp.mult)
            nc.vector.tensor_tensor(out=o3, in0=o3, in1=bcast_feat(scb2[:, D:2 * D]),
                                    op=AluOp.add)
            nc.sync.dma_start(out_d.ap()[:, cols], O[:])

    nc.compile()
    return nc


# ----------------------------------------------------------------------------
# Entry point
# ----------------------------------------------------------------------------

def _assemble(results, layout):
    N = layout["N"]; NS = layout["NS"]; NB = layout["NB"]
    D = layout["D"]; NPAD = layout["NPAD"]; C = layout["C"]
    out = np.empty((N, D), np.float32)
    for c in range(C):
        raw = np.asarray(results[c]["out"]).astype(np.float32)
        srt = raw.reshape(128, NB, D).transpose(1, 0, 2).reshape(NPAD, D)
        out[layout["nids"][c][:NS]] = srt[:NS]
    return out


def _run(inputs, C=N_CORES):
    in_maps, layout = _prep(
        inputs["h"], inputs["ef"], inputs["norm"],
        inputs["gamma1"], inputs["beta1"], inputs["gamma2"], inputs["beta2"],
        inputs["src"], inputs["dst"], C)
    nc = _build(layout)
    res = run_bass_kernel_spmd(nc, in_maps, list(range(C)))
    out = _assemble(res.results, layout)
    return out, res, layout, nc, in_maps


def kernel(**inputs) -> np.ndarray:
    out, _, _, _, _ = _run(inputs)
    return out


# revision 18
# speedup vs baseline: 1.0274x; 1.0274x over previous
"""Trainium2 Bass kernel for nn_ActivationPNALayer (PNA-style GNN message passing).

Strategy (8 NeuronCores, SPMD):
  - 1D graph partition by destination node; nodes are globally sorted by
    in-degree and dealt round-robin across cores (rank r -> core r%8), so every
    core's block b spans the same global degree range -> minimal slot padding
    and perfect load balance.
  - Host-side partitioning/layout: edges laid out as a padded
    [128-node-block x K_b] slot grid in k-OUTER (k, j, d) order; halo features
    h[src]*norm[src] are materialized into a per-edge fp16 stream at partition
    time (ghost-feature materialization).
  - Device: streams hn-slots + ef-slots, forms w = hn[src]+ef (fp16, DVE 2x),
    reduces sum/sumsq/max/min over the slot axis per node via pairwise fp16
    tensor_tensor trees. k-outer layout keeps the packed (j,d) axis innermost
    at every tree level, so every level runs in the DVE 2x fp16 mode
    (tensor_reduce has no fast mode; strided single-column ops are 2-4x
    slower than packed ones). Per-node scalars (1/deg, wn, c1, npad, valid)
    are folded on the host into expanded [node,D] fp16 side streams so the
    tail has no stride-0 broadcast multiplies.
  - Both BatchNorms use PE for the [2 x D] statistics with a cross-core
    AllReduce. Pads duplicate each node's first edge (max/min-safe); sums are
    corrected with npad * w0 where w0 is sliced from slot 0 on device.

Math identity used (the reference's concat/mean tower collapses):
  z_e = w_e + hin[dst],  w_e = hin[src_e] + ef_e
  mean_z = mean_w + hin; max_z = max_w + hin; min_z = min_w + hin; std_z = std_w
  ht = hin*(1/13 + 3*wn) + (mean_w+max_w+min_w+std_w) * wn,
  wn = (1 + logd/A + A/logd)/13
  out = BN2(relu(BN1(ht)) * norm)   [relu(u*norm) = relu(u)*norm since norm > 0]
"""

import os
import sys
from contextlib import ExitStack

import numpy as np

for _p in ("/opt/trn_rl_repo",):
    if os.path.isdir(_p) and _p not in sys.path:
        sys.path.insert(0, _p)

import concourse.bacc as bacc
import concourse.bass as bass
import concourse.mybir as mybir
import concourse.tile as tile
from concourse.bass_utils import run_bass_kernel_spmd

AluOp = mybir.AluOpType
ActFn = mybir.ActivationFunctionType
Axis = mybir.AxisListType
F32 = mybir.dt.float32
F16 = mybir.dt.float16
NPF16 = np.float16

EPS_STD = 1e-5
EPS_BN = 1e-5
AVG_D_LOG = float(np.log(17.0))
N_CORES = 8
FDBUD_DEFAULT = 10240   # max free-dim elements of one slot-stream tile
SQ_ON_GPSIMD = True    # A/B: run the sumsq tree on the Pool engine


# ----------------------------------------------------------------------------
# Host-side partition / layout prep
# ----------------------------------------------------------------------------

def _prep(h, ef, norm, gamma1, beta1, gamma2, beta2, src, dst, C,
          fdbud=FDBUD_DEFAULT):
    h = np.asarray(h, np.float32)
    ef = np.ascontiguousarray(np.asarray(ef, np.float32))
    norm = np.asarray(norm, np.float32).reshape(-1, 1)
    src = np.asarray(src).astype(np.int64)
    dst = np.asarray(dst).astype(np.int64)
    N, D = h.shape
    NS = N // C
    NPAD = ((NS + 127) // 128) * 128
    NB = NPAD // 128
    CH = max(d for d in range(1, min(NB, 49) + 1) if NB % d == 0)
    NCH = NB // CH

    hn = h * norm  # ghost/halo feature table materialized at partition time

    deg = np.bincount(dst, minlength=N).astype(np.int64)
    order = np.argsort(dst, kind="stable")
    starts = np.zeros(N + 1, np.int64)
    starts[1:] = np.cumsum(deg)

    # striped degree-sorted assignment: rank r -> core r%C, position r//C
    grank = np.argsort(-deg, kind="stable")
    nids = np.empty((C, NPAD), np.int64)
    for c in range(C):
        ids = grank[c::C]
        if NPAD > NS:
            ids = np.concatenate([ids, np.full(NPAD - NS, ids[-1], np.int64)])
        nids[c] = ids
    degs_sorted = deg[nids]                       # [C, NPAD]

    Kb = degs_sorted.reshape(C, NB, 128).max(axis=(0, 2))
    Kb = np.maximum(Kb, 1).astype(np.int64)

    # block groups per chunk: uniform K per group, free-dim budget fdbud.
    groups = []          # [NCH][(j0, nb, K, chunk-local coloff)]
    Kb_eff = np.zeros(NB, np.int64)   # actual slots per block (group-uniform)
    TOTF_ch = []
    for ch in range(NCH):
        j = 0
        gl = []
        coloff = 0
        while j < CH:
            K = int(Kb[ch * CH + j])
            nb = int(min(CH - j, max(1, fdbud // (D * K))))
            gl.append((j, nb, K, coloff))
            Kb_eff[ch * CH + j: ch * CH + j + nb] = K
            coloff += nb * D * K
            j += nb
        groups.append(gl)
        TOTF_ch.append(coloff)

    gbv = np.concatenate(
        [np.asarray(x, np.float32).reshape(-1) for x in (gamma1, beta1, gamma2, beta2)]
    ).reshape(1, 4 * D)

    # per-node folded scalars (f64 on host, stored fp16 expanded over D)
    degf = deg.astype(np.float64)
    logd = np.log(degf + 1.0)
    wn_n = (1.0 + logd / AVG_D_LOG + AVG_D_LOG / logd) / 13.0
    c1_n = 1.0 / 13.0 + 3.0 * wn_n
    invd_n = 1.0 / degf

    in_maps = []
    for c in range(C):
        nid_grid = nids[c].reshape(NB, 128)          # [NB, 128] global ids

        # flat per-chunk streams: group g occupies [128*off, 128*(off+F)) as a
        # [128, F] C-order block; within a group the order is (k, j, d) so the
        # packed (j, d) axis stays innermost at every tree level.
        stream = {}
        for ch in range(NCH):
            hns = np.empty(128 * TOTF_ch[ch], NPF16)
            efs = np.empty(128 * TOTF_ch[ch], NPF16)
            for (j0, nb, K, off) in groups[ch]:
                b0 = ch * CH + j0
                n = nid_grid[b0:b0 + nb]                       # [nb, 128]
                s = starts[n][:, :, None]                      # [nb, 128, 1]
                ks = np.arange(K)[None, None, :]
                kk = np.where(ks < deg[n][:, :, None], ks, 0)  # pads -> slot 0
                eids = order[s + kk]                           # [nb, 128, K]
                sidx = src[eids]
                # (nb, 128, K, D) -> (128, K, nb, D)
                hg = hn[sidx].transpose(1, 2, 0, 3).reshape(128, nb * D * K)
                eg = ef[eids].transpose(1, 2, 0, 3).reshape(128, nb * D * K)
                goff = 128 * off
                hns[goff:goff + 128 * nb * D * K] = hg.astype(NPF16).reshape(-1)
                efs[goff:goff + 128 * nb * D * K] = eg.astype(NPF16).reshape(-1)
            stream[f"hns{ch}"] = hns
            stream[f"efs{ch}"] = efs

        meta = nid_grid.T                            # [128, NB]
        sortedidx = np.arange(NPAD).reshape(NB, 128).T
        validf = (sortedidx < NS).astype(np.float64)
        normv = (norm[meta, 0] * validf)             # norm, zeroed for pads
        npadm = (Kb_eff[None, :] - deg[meta]).astype(np.float64)

        def expand(vals128xNB):  # -> [128, NB*D] fp16
            return np.ascontiguousarray(
                np.repeat(vals128xNB[:, :, None], D, axis=2)
                .reshape(128, NB * D).astype(NPF16))

        # hselfC1 = hn[node]*c1*valid  [128, NB*D]
        hselfC1 = (hn[nid_grid] * (c1_n[nid_grid] * validf.T)[:, :, None]) \
            .transpose(1, 0, 2).reshape(128, NB * D)

        in_maps.append(dict(
            **stream,
            hselfc1=np.ascontiguousarray(hselfC1.astype(NPF16)),
            wnd=expand(wn_n[meta] * validf),
            invd=expand(invd_n[meta]),
            npd=expand(npadm),
            nrmd=expand(normv),
            gbv=gbv,
        ))

    layout = dict(N=N, D=D, C=C, NS=NS, NPAD=NPAD, NB=NB, CH=CH, NCH=NCH,
                  groups=groups, TOTF_ch=TOTF_ch, Kb=[int(k) for k in Kb],
                  nids=nids)
    return in_maps, layout


# ----------------------------------------------------------------------------
# Device program
# ----------------------------------------------------------------------------

def _build(layout):
    N = layout["N"]; D = layout["D"]; C = layout["C"]
    NB = layout["NB"]; CH = layout["CH"]; NCH = layout["NCH"]
    groups = layout["groups"]; TOTF_ch = layout["TOTF_ch"]

    nc = bacc.Bacc("TRN2", target_bir_lowering=False, debug=False,
                   enable_asserts=False, num_devices=C)

    hns_d = [nc.dram_tensor(f"hns{ch}", [128 * TOTF_ch[ch]], F16,
                            kind="ExternalInput") for ch in range(NCH)]
    efs_d = [nc.dram_tensor(f"efs{ch}", [128 * TOTF_ch[ch]], F16,
                            kind="ExternalInput") for ch in range(NCH)]
    hselfc1_d = nc.dram_tensor("hselfc1", [128, NB * D], F16, kind="ExternalInput")
    wnd_d = nc.dram_tensor("wnd", [128, NB * D], F16, kind="ExternalInput")
    invd_d = nc.dram_tensor("invd", [128, NB * D], F16, kind="ExternalInput")
    npd_d = nc.dram_tensor("npd", [128, NB * D], F16, kind="ExternalInput")
    nrmd_d = nc.dram_tensor("nrmd", [128, NB * D], F16, kind="ExternalInput")
    gbv_d = nc.dram_tensor("gbv", [1, 4 * D], F32, kind="ExternalInput")
    out_d = nc.dram_tensor("out", [128, NB * D], F16, kind="ExternalOutput")

    rg = [list(range(C))]

    with tile.TileContext(nc) as tc, ExitStack() as stack:
        res = stack.enter_context(tc.tile_pool(name="res", bufs=1))
        slotp = stack.enter_context(tc.tile_pool(name="slot", bufs=2))
        scrp = stack.enter_context(tc.tile_pool(name="scr", bufs=1))
        chkp = stack.enter_context(tc.tile_pool(name="chk", bufs=2))
        tailp = stack.enter_context(tc.tile_pool(name="tail", bufs=1))
        psp = stack.enter_context(tc.tile_pool(name="ps", bufs=1, space="PSUM"))
        drp = stack.enter_context(tc.tile_pool(name="dr", bufs=1, space="DRAM"))

        # resident tiles
        gb_t = res.tile([1, 4 * D], F32)
        nc.sync.dma_start(gb_t[:], gbv_d.ap())

        ones_colf = res.tile([128, 1], F32)
        ones_row = res.tile([1, 128], F32)
        eps_t = res.tile([128, 1], F32)
        nc.vector.memset(ones_colf[:], 1.0)
        nc.vector.memset(ones_row[:], 1.0)
        nc.vector.memset(eps_t[:], EPS_STD)

        HT = res.tile([128, NB * D], F16)     # ht, then y in place

        # warm up the collective path early (hidden under phase-1 streaming)
        # so the two real AllReduces run at steady-state latency (~9us vs ~37us)
        wi = drp.tile([1, 2 * D], F32, tag="warmin")
        wo = drp.tile([1, 2 * D], F32, tag="warmout")
        nc.sync.dma_start(wi[:], gb_t[:, 0:2 * D])
        nc.gpsimd.collective_compute(
            "AllReduce", AluOp.add, replica_groups=rg,
            ins=[wi[:].opt()], outs=[wo[:].opt()])



        def bcast_node(ap128xCH):
            return ap128xCH.to_broadcast([128, CH, D])

        def bcast_feat(ap128xD):
            return ap128xD.rearrange("p (o d) -> p o d", o=1).to_broadcast([128, CH, D])

        def bn_stats_chunk(htc, pbn, first):
            """Per-chunk BN partial: fp16 trees over the CH chunk columns,
            accumulated into [128, 2D] f32; one AllReduce per BN happens at
            the phase boundary (mid-phase collectives block the gpsimd queue
            and can cascade into DVE stalls)."""
            SQH = tailp.tile([128, CH * D], F16, tag="npdc")
            nc.scalar.activation(out=SQH[:], in_=htc, func=ActFn.Square)
            ht3 = htc.rearrange("p (c d) -> p c d", c=CH)
            sq3 = SQH[:].rearrange("p (c d) -> p c d", c=CH)
            Tb = tailp.tile([128, (CH // 2) * D], F16, tag="bntree")
            Tb3 = Tb[:].rearrange("p (c d) -> p c d", c=CH // 2)
            if first:
                tree(nc.vector, ht3, CH, AluOp.add,
                     pbn[:, 0:D].rearrange("p (c d) -> p c d", c=1), Tb3)
                tree(nc.vector, sq3, CH, AluOp.add,
                     pbn[:, D:2 * D].rearrange("p (c d) -> p c d", c=1), sq3)
            else:
                pacc = tailp.tile([128, 2 * D], F32, tag="pacc")
                tree(nc.vector, ht3, CH, AluOp.add,
                     pacc[:, 0:D].rearrange("p (c d) -> p c d", c=1), Tb3)
                tree(nc.vector, sq3, CH, AluOp.add,
                     pacc[:, D:2 * D].rearrange("p (c d) -> p c d", c=1), sq3)
                nc.vector.tensor_tensor(out=pbn[:], in0=pbn[:], in1=pacc[:],
                                        op=AluOp.add)

        def tree(eng, cur3, s, op, acc3, Tv):
            """Pairwise reduce [128, s, nbD] over axis 1 into acc3
            ([128, 1, nbD]); Tv is the scratch view [128, s//2, nbD] (may
            alias cur3's buffer for in-place trees). k-outer layout keeps the
            innermost axis packed at every level -> DVE 2x fp16 mode."""
            if s == 1:
                nc.vector.tensor_copy(out=acc3, in_=cur3[:, 0:1, :])
                return
            while s > 1:
                hh = s // 2
                out = acc3 if hh == 1 else Tv[:, 0:hh, :]
                eng.tensor_tensor(out=out, in0=cur3[:, 0:hh, :],
                                  in1=cur3[:, hh:2 * hh, :], op=op)
                if s % 2:
                    eng.tensor_tensor(out=out[:, 0:1, :], in0=out[:, 0:1, :],
                                      in1=cur3[:, 2 * hh:s, :], op=op)
                cur3 = out
                s = hh

        # ================= phase 1: stream slots, reduce, tower =================
        pbn1 = res.tile([128, 2 * D], F32, name="pbn1")
        pbn2 = res.tile([128, 2 * D], F32, name="pbn2")
        for ch in range(NCH):
            cols = slice(ch * CH * D, (ch + 1) * CH * D)
            colsn = slice(ch * CH, (ch + 1) * CH)
            sumC = chkp.tile([128, CH * D], F16, tag="sumC")
            maxC = chkp.tile([128, CH * D], F16, tag="maxC")
            minC = chkp.tile([128, CH * D], F16, tag="minC")
            sqC = chkp.tile([128, CH * D], F16, tag="sqC")
            w0c = chkp.tile([128, CH * D], F16, tag="w0c")

            for (j0, nb, K, off) in groups[ch]:
                F = nb * D * K
                goff = 128 * off
                ef_view = efs_d[ch].ap()[goff:goff + 128 * F].rearrange(
                    "(p f) -> p f", p=128)
                hn_view = hns_d[ch].ap()[goff:goff + 128 * F].rearrange(
                    "(p f) -> p f", p=128)
                W = slotp.tile([128, F], F16, tag="w")
                nc.sync.dma_start(W[:], ef_view)
                if CCE_ADD:
                    inst = nc.sync.dma_start(W[:], hn_view)
                    inst.ins.cce_op = AluOp.add
                    SQT = slotp.tile([128, F], F16, tag="hn")
                else:
                    HNS = slotp.tile([128, F], F16, tag="hn")
                    nc.sync.dma_start(HNS[:], hn_view)
                    nc.vector.tensor_tensor(out=W[:], in0=W[:], in1=HNS[:],
                                            op=AluOp.add)
                    SQT = HNS
                Wv = W[:].rearrange("p (k f) -> p k f", k=K)
                osl = slice(j0 * D, (j0 + nb) * D)

                def acc3(t):
                    return t[:, osl].rearrange("p (k f) -> p k f", k=1)

                # w0 slice (first real edge) on Act; SQ overwrites HNS buffer
                nc.scalar.activation(out=acc3(w0c), in_=Wv[:, 0:1, :],
                                     func=ActFn.Copy)
                nc.scalar.activation(out=SQT[:], in_=W[:], func=ActFn.Square)
                SQv = SQT[:].rearrange("p (k f) -> p k f", k=K)

                if K > 1:
                    T = scrp.tile([128, nb * D * (K // 2)], F16, tag="t")
                    Tv = T[:].rearrange("p (k f) -> p k f", k=K // 2)
                else:
                    Tv = None
                tree(nc.vector, Wv, K, AluOp.add, acc3(sumC), Tv)
                tree(nc.vector, Wv, K, AluOp.max, acc3(maxC), Tv)
                tree(nc.vector, Wv, K, AluOp.min, acc3(minC), Tv)
                sq_eng = nc.gpsimd if SQ_ON_GPSIMD else nc.vector
                tree(sq_eng, SQv, K, AluOp.add, acc3(sqC), SQv)

            # ---- chunk tail (all packed fp16, no broadcasts) ----
            hsc = tailp.tile([128, CH * D], F16, tag="hsc")
            nc.sync.dma_start(hsc[:], hselfc1_d.ap()[:, cols])
            wndc = tailp.tile([128, CH * D], F16, tag="wndc")
            nc.sync.dma_start(wndc[:], wnd_d.ap()[:, cols])
            invdc = tailp.tile([128, CH * D], F16, tag="invdc")
            nc.sync.dma_start(invdc[:], invd_d.ap()[:, cols])
            npdc = tailp.tile([128, CH * D], F16, tag="npdc")
            nc.sync.dma_start(npdc[:], npd_d.ap()[:, cols])

            tmp = tailp.tile([128, CH * D], F16, tag="tmp")

            # mean = (sumC - npd*w0)*invd  (into sumC)
            nc.vector.tensor_tensor(out=tmp[:], in0=w0c[:], in1=npdc[:],
                                    op=AluOp.mult)
            nc.vector.tensor_tensor(out=sumC[:], in0=sumC[:], in1=tmp[:],
                                    op=AluOp.subtract)
            nc.vector.tensor_tensor(out=sumC[:], in0=sumC[:], in1=invdc[:],
                                    op=AluOp.mult)
            # ex2 = (sqC - npd*w0^2)*invd  (into sqC); npd*w0^2 = tmp*w0
            nc.vector.tensor_tensor(out=tmp[:], in0=tmp[:], in1=w0c[:],
                                    op=AluOp.mult)
            nc.vector.tensor_tensor(out=sqC[:], in0=sqC[:], in1=tmp[:],
                                    op=AluOp.subtract)
            nc.vector.tensor_tensor(out=sqC[:], in0=sqC[:], in1=invdc[:],
                                    op=AluOp.mult)
            # std = sqrt(relu(ex2 - mean^2) + eps)  (into sqC)
            nc.scalar.activation(out=tmp[:], in_=sumC[:], func=ActFn.Square)
            nc.vector.tensor_tensor(out=sqC[:], in0=sqC[:], in1=tmp[:],
                                    op=AluOp.subtract)
            nc.scalar.activation(out=sqC[:], in_=sqC[:], func=ActFn.Relu)
            nc.scalar.activation(out=sqC[:], in_=sqC[:], func=ActFn.Sqrt,
                                 bias=eps_t[:])

            # S = mean + max + min + std (into maxC)
            nc.vector.tensor_tensor(out=maxC[:], in0=maxC[:], in1=minC[:], op=AluOp.add)
            nc.vector.tensor_tensor(out=maxC[:], in0=maxC[:], in1=sumC[:], op=AluOp.add)
            nc.vector.tensor_tensor(out=maxC[:], in0=maxC[:], in1=sqC[:], op=AluOp.add)

            # ht = hself*c1*valid + S*wn*valid
            htc = HT[:, cols]
            nc.vector.tensor_tensor(out=maxC[:], in0=maxC[:], in1=wndc[:],
                                    op=AluOp.mult)
            nc.vector.tensor_tensor(out=htc, in0=hsc[:], in1=maxC[:], op=AluOp.add)
            bn_stats_chunk(htc, pbn1, ch == 0)

        # ================= BN finalize helper =================
        def bn_finalize(pbn, g_off, b_off, cc_tag):
            pcmb = psp.tile([1, 2 * D], F32, tag="pscmb", space="PSUM")
            nc.tensor.matmul(out=pcmb[:, 0:D], lhsT=ones_colf[:],
                             rhs=pbn[:, 0:D], start=True, stop=True)
            nc.tensor.matmul(out=pcmb[:, D:2 * D], lhsT=ones_colf[:],
                             rhs=pbn[:, D:2 * D], start=True, stop=True)
            stats = res.tile([1, 2 * D], F32, tag=f"st{cc_tag}")
            nc.vector.tensor_copy(out=stats[:], in_=pcmb[:])
            cin = drp.tile([1, 2 * D], F32, tag=f"cin{cc_tag}")
            cout = drp.tile([1, 2 * D], F32, tag=f"cout{cc_tag}")
            nc.sync.dma_start(cin[:], stats[:])
            nc.gpsimd.collective_compute(
                "AllReduce", AluOp.add, replica_groups=rg,
                ins=[cin[:].opt()], outs=[cout[:].opt()])
            ar = res.tile([1, 2 * D], F32, tag=f"ar{cc_tag}")
            nc.sync.dma_start(ar[:], cout[:])
            mean = res.tile([1, D], F32, tag=f"mean{cc_tag}")
            nc.vector.tensor_scalar(out=mean[:], in0=ar[:, 0:D], scalar1=1.0 / N,
                                    scalar2=None, op0=AluOp.mult)
            var = res.tile([1, D], F32, tag=f"var{cc_tag}")
            nc.vector.tensor_scalar(out=var[:], in0=ar[:, D:2 * D], scalar1=1.0 / N,
                                    scalar2=None, op0=AluOp.mult)
            msq = res.tile([1, D], F32, tag=f"msq{cc_tag}")
            nc.scalar.activation(out=msq[:], in_=mean[:], func=ActFn.Square)
            nc.vector.tensor_tensor(out=var[:], in0=var[:], in1=msq[:],
                                    op=AluOp.subtract)
            nc.scalar.activation(out=var[:], in_=var[:], func=ActFn.Sqrt,
                                 bias=eps_t[:1, :])
            rstd = res.tile([1, D], F32, tag=f"rstd{cc_tag}")
            nc.vector.reciprocal(rstd[:], var[:])
            sc = res.tile([1, 2 * D], F32, tag=f"sc{cc_tag}")
            nc.vector.tensor_tensor(out=sc[:, 0:D], in0=gb_t[:, g_off:g_off + D],
                                    in1=rstd[:], op=AluOp.mult)
            nc.vector.tensor_tensor(out=sc[:, D:2 * D], in0=mean[:], in1=sc[:, 0:D],
                                    op=AluOp.mult)
            nc.vector.tensor_tensor(out=sc[:, D:2 * D], in0=gb_t[:, b_off:b_off + D],
                                    in1=sc[:, D:2 * D], op=AluOp.subtract)
            psb = psp.tile([128, 2 * D], F32, tag="psbc", space="PSUM")
            nc.tensor.matmul(out=psb[:], lhsT=ones_row[:], rhs=sc[:], start=True,
                             stop=True)
            scb = res.tile([128, 2 * D], F16, tag=f"scb{cc_tag}")
            nc.vector.tensor_copy(out=scb[:], in_=psb[:])
            return scb

        scb1 = bn_finalize(pbn1, 0, D, "1")

        # ================= phase 2: y = relu(BN1(ht)) * norm =================
        for ch in range(NCH):
            cols = slice(ch * CH * D, (ch + 1) * CH * D)
            colsn = slice(ch * CH, (ch + 1) * CH)
            htc = HT[:, cols]
            ht3 = htc.rearrange("p (c d) -> p c d", c=CH)
            U = tailp.tile([128, CH * D], F16, tag="tmp")
            u3 = U[:].rearrange("p (c d) -> p c d", c=CH)
            nc.vector.tensor_tensor(out=u3, in0=ht3, in1=bcast_feat(scb1[:, 0:D]),
                                    op=AluOp.mult)
            nc.vector.tensor_tensor(out=u3, in0=u3, in1=bcast_feat(scb1[:, D:2 * D]),
                                    op=AluOp.add)
            nc.scalar.activation(out=U[:], in_=U[:], func=ActFn.Relu)
            # nrmd = norm * valid expanded (host): zeroes pad nodes for BN2 stats
            nrc = tailp.tile([128, CH * D], F16, tag="invdc")
            nc.sync.dma_start(nrc[:], nrmd_d.ap()[:, cols])
            nc.vector.tensor_tensor(out=htc, in0=U[:], in1=nrc[:],
                                    op=AluOp.mult)
            bn_stats_chunk(htc, pbn2, ch == 0)

        scb2 = bn_finalize(pbn2, 2 * D, 3 * D, "2")

        # ================= phase 3: out = BN2(y) =================
        for ch in range(NCH):
            cols = slice(ch * CH * D, (ch + 1) * CH * D)
            htc = HT[:, cols]
            ht3 = htc.rearrange("p (c d) -> p c d", c=CH)
            O = tailp.tile([128, CH * D], F16, tag="tmp")
            o3 = O[:].rearrange("p (c d) -> p c d", c=CH)
            nc.vector.tensor_tensor(out=o3, in0=ht3, in1=bcast_feat(scb2[:, 0:D]),
                                    op=AluOp.mult)
            nc.vector.tensor_tensor(out=o3, in0=o3, in1=bcast_feat(scb2[:, D:2 * D]),
                                    op=AluOp.add)
            nc.sync.dma_start(out_d.ap()[:, cols], O[:])

    nc.compile()
    return nc


# ----------------------------------------------------------------------------
# Entry point
# ----------------------------------------------------------------------------

def _assemble(results, layout):
    N = layout["N"]; NS = layout["NS"]; NB = layout["NB"]
    D = layout["D"]; NPAD = layout["NPAD"]; C = layout["C"]
    out = np.empty((N, D), np.float32)
    for c in range(C):
        raw = np.asarray(results[c]["out"]).astype(np.float32)
        srt = raw.reshape(128, NB, D).transpose(1, 0, 2).reshape(NPAD, D)
        out[layout["nids"][c][:NS]] = srt[:NS]
    return out


def _run(inputs, C=N_CORES):
    in_maps, layout = _prep(
        inputs["h"], inputs["ef"], inputs["norm"],
        inputs["gamma1"], inputs["beta1"], inputs["gamma2"], inputs["beta2"],
        inputs["src"], inputs["dst"], C)
    nc = _build(layout)
    res = run_bass_kernel_spmd(nc, in_maps, list(range(C)))
    out = _assemble(res.results, layout)
    return out, res, layout, nc, in_maps


def kernel(**inputs) -> np.ndarray:
    out, _, _, _, _ = _run(inputs)
    return out
